# revision 1
# baseline (speedup 1.0000x reference)
"""HELMo encoder (bi-GRU over 3 steps + MHA + classifier) on 8 trn2 cores.

Data-parallel over batch (8192 -> 8 x 1024). Per core, one Bass/Tile kernel:
  A) fused GRU in bf16: input+hidden projections accumulate into f32 PSUM
     (k = [x; h_prev] vs W_cat = [W_ih.T; W_hh.T]); gates split across
     ACT (sigmoid/tanh), DVE (psum-side adds/muls) and Pool (SBUF-side
     combine); h stored bf16, feature-major.
  B) fused attention pass, one 128-row batch chunk at a time, all-SBUF:
     - Q/K projections as fp8e4 DoubleRow matmuls (2x PE rate); weights
       pre-scaled x16, hs x4 (exact powers of 2, fp8 normal range), with
       the compensation folded into the softmax exp scale; q/k stored
       fp8 so all chunk pools double-buffer.
     - V projection in bf16, W_v streamed in 128-col pieces, each
       shared across a pair of batch chunks (halves re-streaming).
     - per-head logits / softmax / (sum_tq w) combine on DVE; every AP
       kept at <=2 free dims (3-free-dim views on ACT cost ~75us/op on
       real hardware).
     - W_o folded into the classifier on the host (W_eff = W_out @ W_o).
     - the PE-after-DVE tail (ctx transpose + classifier + softmax) is
       deferred to one sweep after all 8 chunks, keeping the PE stream
       unbroken matmuls (p-state stays at full clock); phase-B weight
       DMAs are hoisted above phase A to overlap the GRU; GRU h-state
       stores issue from the ACT engine's DGE ring so the SP ring
       serves only latency-critical loads.

Matmul accumulation is f32 PSUM throughout; rel err vs f32 reference
~5e-3, dominated by bf16 weight quantization.
"""

import os
import sys

sys.path.insert(0, "/opt/trn_rl_repo")

import numpy as np

import concourse.bacc as bacc
import concourse.bass as bass
import concourse.mybir as mybir
import concourse.tile as tile
from concourse.masks import make_identity

dt = mybir.dt
AF = mybir.ActivationFunctionType
AX = mybir.AxisListType
PM = mybir.MatmulPerfMode

N_CORES = 8
B = 8192
B_LOC = B // N_CORES          # 1024
I = 1024
H = 1024
D = 2 * H                     # 2048
NH = 16
HD = 128
S = 3
C = 7
P = 128
HJT = H // P                  # 8 jtiles per gate
KC_D = D // P                 # 16
NCH = B_LOC // P              # 8 batch chunks in the attention pass

W8_SCALE = 16.0               # fp8 weight pre-scale (host side)
H8_SCALE = 4.0                # fp8 activation pre-scale (device side)
EXP_SCALE = 1.0 / (W8_SCALE * W8_SCALE * H8_SCALE * H8_SCALE * HD ** 0.5)
USE_DOUBLE_ROW = True

f32, bf, f8 = dt.float32, dt.bfloat16, dt.float8e4

_CACHE = {}


def _r3(ap, pat, **kw):
    return ap.rearrange(pat, **kw)


def _gru_cell(nc, pps, gpool, tpool, bias_t, j, bs, first,
              x_chunk, h_prev, h_new, wr, wz, wn):
    t_brz, t_nbz, t_bni, t_bnh = bias_t
    if first:
        pz = pps.tile([P, 512], f32, tag="pz")
        for c in range(HJT):
            nc.tensor.matmul(pz[:], wz[:, c, :], x_chunk(c)[:, bs],
                             start=(c == 0), stop=(c == HJT - 1))
        pgi = pps.tile([P, 512], f32, tag="pgi")
        for c in range(HJT):
            nc.tensor.matmul(pgi[:], wn[:, c, :], x_chunk(c)[:, bs],
                             start=(c == 0), stop=(c == HJT - 1))
        zc = gpool.tile([P, 512], f32, tag="z")
        nc.scalar.activation(zc[:], pz[:], AF.Sigmoid,
                             bias=t_nbz[:, j, :], scale=-1.0)
        n_sb = gpool.tile([P, 512], f32, tag="n")
        nc.scalar.activation(n_sb[:], pgi[:], AF.Tanh, bias=t_bni[:, j, :])
        nc.gpsimd.tensor_mul(h_new[:, j, bs], zc[:], n_sb[:])
        return

    nk = 2 * HJT

    def mm_acc(ptile, ws):
        for c in range(nk):
            rhs = (x_chunk(c)[:, bs] if c < HJT else h_prev[:, c - HJT, bs])
            nc.tensor.matmul(ptile[:], ws[:, c, :], rhs,
                             start=(c == 0), stop=(c == nk - 1))

    pr = pps.tile([P, 512], f32, tag="pr")
    mm_acc(pr, wr)
    pz = pps.tile([P, 512], f32, tag="pz")
    mm_acc(pz, wz)
    pgi = pps.tile([P, 512], f32, tag="pgi")
    for c in range(HJT):
        nc.tensor.matmul(pgi[:], wn[:, c, :], x_chunk(c)[:, bs],
                         start=(c == 0), stop=(c == HJT - 1))
    pgh = pps.tile([P, 512], f32, tag="pgh")
    for c in range(HJT, 2 * HJT):
        nc.tensor.matmul(pgh[:], wn[:, c, :], h_prev[:, c - HJT, bs],
                         start=(c == HJT), stop=(c == 2 * HJT - 1))
    r_sb = gpool.tile([P, 512], f32, tag="r")
    nc.scalar.activation(r_sb[:], pr[:], AF.Sigmoid, bias=t_brz[:, j, :])
    z_sb = gpool.tile([P, 512], f32, tag="z")
    nc.scalar.activation(z_sb[:], pz[:], AF.Sigmoid, bias=t_brz[:, HJT + j, :])
    t1 = tpool.tile([P, 512], f32, tag="tmp")
    nc.vector.tensor_scalar_add(t1[:], pgh[:], t_bnh[:, j, :])
    t2 = tpool.tile([P, 512], f32, tag="tmp")
    nc.vector.tensor_mul(t2[:], r_sb[:], t1[:])
    t3 = tpool.tile([P, 512], f32, tag="tmp")
    nc.vector.tensor_add(t3[:], pgi[:], t2[:])
    n_sb = gpool.tile([P, 512], f32, tag="n")
    nc.scalar.activation(n_sb[:], t3[:], AF.Tanh, bias=t_bni[:, j, :])
    t4 = tpool.tile([P, 512], bf, tag="tmpb")
    nc.gpsimd.tensor_sub(t4[:], h_prev[:, j, bs], n_sb[:])
    t5 = tpool.tile([P, 512], bf, tag="tmpb")
    nc.gpsimd.tensor_mul(t5[:], z_sb[:], t4[:])
    nc.gpsimd.tensor_add(h_new[:, j, bs], t5[:], n_sb[:])


def _phase_a(nc, tc, T, hs_bf):
    with (tc.tile_pool(name="ga_const", bufs=1) as cpool,
          tc.tile_pool(name="ga_x", bufs=3) as xpool,
          tc.tile_pool(name="ga_h", bufs=3) as hpool,
          tc.tile_pool(name="ga_w", bufs=2) as wpool,
          tc.tile_pool(name="ga_g", bufs=2) as gpool,
          tc.tile_pool(name="ga_t", bufs=3) as tpool,
          tc.tile_pool(name="ga_ps", bufs=2, space="PSUM") as pps):
        bias = {}
        for d in ("f", "b"):
            t_brz = cpool.tile([P, 2 * HJT, 1], f32, tag=f"brz{d}")
            nc.sync.dma_start(t_brz[:], _r3(T[f"brz_{d}"][:], "(c k) o -> k c o", k=P))
            t_nbz = cpool.tile([P, HJT, 1], f32, tag=f"nbz{d}")
            nc.sync.dma_start(t_nbz[:], _r3(T[f"negbz_{d}"][:], "(c k) o -> k c o", k=P))
            t_bni = cpool.tile([P, HJT, 1], f32, tag=f"bni{d}")
            nc.sync.dma_start(t_bni[:], _r3(T[f"bnih_{d}"][:], "(c k) o -> k c o", k=P))
            t_bnh = cpool.tile([P, HJT, 1], f32, tag=f"bnh{d}")
            nc.sync.dma_start(t_bnh[:], _r3(T[f"bnhh_{d}"][:], "(c k) o -> k c o", k=P))
            bias[d] = (t_brz, t_nbz, t_bni, t_bnh)

        order = [(0, "f", 0), (0, "b", 2), (1, "f", 1),
                 (1, "b", 1), (2, "f", 2), (2, "b", 0)]
        h_cur = {"f": None, "b": None}
        for step, d, t in order:
            first = step == 0
            x_halves = []
            for xh in range(2):
                xv = xpool.tile([P, HJT // 2, B_LOC], bf, tag="x",
                                name=f"x_{step}_{d}_{xh}")
                nc.sync.dma_start(
                    xv[:],
                    _r3(T["xt"][t][xh * (I // 2):(xh + 1) * (I // 2), :],
                        "(c k) b -> k c b", k=P))
                x_halves.append(xv)

            def x_chunk(c):
                return x_halves[c // (HJT // 2)][:, c % (HJT // 2), :]

            h_prev = h_cur[d]
            h_new = hpool.tile([P, HJT, B_LOC], bf, tag="h",
                               name=f"h_{step}_{d}")
            for j in range(HJT):
                # host pre-permutes wcat columns: per j the r/z/n gate
                # columns are adjacent -> one contiguous 384-col DMA
                nkc = HJT if first else 2 * HJT
                wj = wpool.tile([P, nkc, 3 * P], bf, tag="wj",
                                name=f"wj_{step}_{d}_{j}")
                nc.sync.dma_start(
                    wj[:],
                    _r3(T[f"wcat_{d}"][:nkc * P, j * 3 * P:(j + 1) * 3 * P],
                        "(c k) m -> k c m", k=P))
                wr = wj[:, :, 0:P]
                wz = wj[:, :, P:2 * P]
                wn = wj[:, :, 2 * P:3 * P]
                for bt in range(2):
                    bs = slice(bt * 512, (bt + 1) * 512)
                    _gru_cell(nc, pps, gpool, tpool, bias[d], j, bs, first,
                              x_chunk, h_prev, h_new, wr, wz, wn)
                    row = (0 if d == "f" else H) + j * P
                    nc.scalar.dma_start(hs_bf[t, row:row + P, bs],
                                        h_new[:, j, bs])
            h_cur[d] = h_new


def _attn_qk(nc, pools, consts, T, hs_bf, ci):
    (hpool, h8pool, wvpool, qkpool, vpool, appool, smpool, tmpool, ctxpool,
     cmpool, epool, pps_qk, pps_v, pps_t, pps_c) = pools
    wq8_sb, wk8_sb, weff_sb, bout_sb, ident = consts
    bsl = slice(ci * P, (ci + 1) * P)
    hst, hst8 = [], []
    for t in range(S):
        ht = hpool.tile([P, KC_D, P], bf, tag=f"hst{t}", name=f"hst{t}_{ci}")
        nc.sync.dma_start(ht[:], _r3(hs_bf[t][:, bsl], "(c k) b -> k c b", k=P))
        h8 = h8pool.tile([P, KC_D, P], f8, tag=f"h8_{t}", name=f"h8_{t}_{ci}")
        nc.gpsimd.tensor_scalar_mul(h8[:], ht[:], H8_SCALE)
        hst.append(ht)
        hst8.append(h8)

    qt, kt = [], []
    for t in range(S):
        for wsb, lst, nm in ((wq8_sb, qt, "q"), (wk8_sb, kt, "k")):
            osb = qkpool.tile([P, D], f8, tag=f"{nm}{t}",
                              name=f"{nm}{t}_{ci}")
            for do_ in range(4):
                po = pps_qk.tile([P, 512], f32, tag="pqk",
                                 name=f"p{nm}_{t}_{ci}_{do_}")
                if USE_DOUBLE_ROW:
                    for c in range(0, KC_D, 2):
                        nc.tensor.matmul(
                            po[:], hst8[t][:, c:c + 2, :],
                            wsb[:, c:c + 2, do_ * 512:(do_ + 1) * 512],
                            start=(c == 0), stop=(c == KC_D - 2),
                            perf_mode=PM.DoubleRow)
                else:
                    for c in range(KC_D):
                        nc.tensor.matmul(
                            po[:], hst8[t][:, c, :],
                            wsb[:, c, do_ * 512:(do_ + 1) * 512],
                            start=(c == 0), stop=(c == KC_D - 1))
                dsl = slice(do_ * 512, (do_ + 1) * 512)
                nc.scalar.copy(osb[:, dsl], po[:])
            lst.append(osb)

    vt = [vpool.tile([P, D], bf, tag=f"v{t}", name=f"v{t}_{ci}")
          for t in range(S)]
    return {"ci": ci, "hst": hst, "qt": qt, "kt": kt, "vt": vt}


def _attn_v_pair(nc, pools, T, datas):
    """One W_v piece load serves both chunks of the pair."""
    (hpool, h8pool, wvpool, qkpool, vpool, appool, smpool, tmpool, ctxpool,
     cmpool, epool, pps_qk, pps_v, pps_t, pps_c) = pools
    for do_ in range(16):
        wvt = wvpool.tile([P, KC_D, 128], bf, tag="wv",
                          name=f"wv_{datas[0]['ci']}_{do_}")
        nc.sync.dma_start(
            wvt[:],
            _r3(T["wv"][:, do_ * 128:(do_ + 1) * 128], "(c k) m -> k c m", k=P))
        dsl = slice(do_ * 128, (do_ + 1) * 128)
        for d_ in datas:
            for t in range(S):
                pv = pps_v.tile([P, 128], f32, tag="pv",
                                name=f"pv_{d_['ci']}_{do_}_{t}")
                for c in range(KC_D):
                    nc.tensor.matmul(pv[:], d_["hst"][t][:, c, :],
                                     wvt[:, c, :],
                                     start=(c == 0), stop=(c == KC_D - 1))
                nc.scalar.copy(d_["vt"][t][:, dsl], pv[:])


def _attn_softmax(nc, pools, data):
    (hpool, h8pool, wvpool, qkpool, vpool, appool, smpool, tmpool, ctxpool,
     cmpool, epool, pps_qk, pps_v, pps_t, pps_c) = pools
    ci, qt, kt, vt = data["ci"], data["qt"], data["kt"], data["vt"]
    L = smpool.tile([P, NH, S, S], f32, tag="L")
    for tq in range(S):
        for tk in range(S):
            pr_ = appool.tile([P, D], bf, tag="prod", name=f"prod_{ci}_{tq}_{tk}")
            nc.vector.tensor_mul(pr_[:], qt[tq][:], kt[tk][:])
            nc.vector.reduce_sum(
                L[:, :, tq, tk], _r3(pr_[:], "p (h e) -> p h e", h=NH), axis=AX.X)
    # logits are O(0.1) after descaling: exp cannot overflow, skip max-sub.
    # All APs here are kept at <=2 free dims: 3-free-dim views on ACT/DVE
    # are catastrophically slow on hardware (~75us for a [P,16,3,3] exp).
    E2 = smpool.tile([P, NH, S, S], f32, tag="E2")
    nc.scalar.activation(_r3(E2[:], "p h q k -> p (h q k)"),
                         _r3(L[:], "p h q k -> p (h q k)"),
                         AF.Exp, scale=EXP_SCALE)
    Ssum = smpool.tile([P, NH, S], f32, tag="Ssum")
    nc.vector.reduce_sum(_r3(Ssum[:], "p h q -> p (h q)"),
                         _r3(E2[:], "p h q k -> p (h q) k"), axis=AX.X)
    Rs = smpool.tile([P, NH, S], f32, tag="Rs")
    nc.vector.reciprocal(_r3(Rs[:], "p h q -> p (h q)"),
                         _r3(Ssum[:], "p h q -> p (h q)"))
    Wn = smpool.tile([P, NH, S, S], f32, tag="Wn")
    rs_flat = _r3(Rs[:], "p h q -> p (h q)")
    nc.vector.tensor_mul(_r3(Wn[:], "p h q k -> p (h q) k"),
                         _r3(E2[:], "p h q k -> p (h q) k"),
                         rs_flat[:, :, None].broadcast_to([P, NH * S, S]))
    wsum = smpool.tile([P, NH, S], f32, tag="wsum")
    wsA = smpool.tile([P, NH, S], f32, tag="wsA")
    nc.vector.tensor_add(wsA[:], Wn[:, :, 0, :], Wn[:, :, 1, :])
    nc.vector.tensor_add(wsum[:], wsA[:], Wn[:, :, 2, :])

    tm0 = tmpool.tile([P, D], bf, tag="tm0", name=f"tm0_{ci}")
    nc.vector.tensor_mul(
        _r3(tm0[:], "p (h e) -> p h e", h=NH),
        _r3(vt[0][:], "p (h e) -> p h e", h=NH),
        wsum[:, :, 0][:, :, None].broadcast_to([P, NH, HD]))
    tm1 = tmpool.tile([P, D], bf, tag="tm1", name=f"tm1_{ci}")
    nc.vector.tensor_mul(
        _r3(tm1[:], "p (h e) -> p h e", h=NH),
        _r3(vt[1][:], "p (h e) -> p h e", h=NH),
        wsum[:, :, 1][:, :, None].broadcast_to([P, NH, HD]))
    nc.vector.tensor_add(tm0[:], tm0[:], tm1[:])
    tm2 = tmpool.tile([P, D], bf, tag="tm1", name=f"tm2_{ci}")
    nc.vector.tensor_mul(
        _r3(tm2[:], "p (h e) -> p h e", h=NH),
        _r3(vt[2][:], "p (h e) -> p h e", h=NH),
        wsum[:, :, 2][:, :, None].broadcast_to([P, NH, HD]))
    ctx = ctxpool.tile([P, D], bf, tag="ctx", name=f"ctx_{ci}")
    nc.vector.tensor_add(ctx[:], tm0[:], tm2[:])
    return ctx


def _attn_tail(nc, pools, consts, T, ci, ctx):
    (hpool, h8pool, wvpool, qkpool, vpool, appool, smpool, tmpool, ctxpool,
     cmpool, epool, pps_qk, pps_v, pps_t, pps_c) = pools
    wq8_sb, wk8_sb, weff_sb, bout_sb, ident = consts
    bsl = slice(ci * P, (ci + 1) * P)
    ctxm = cmpool.tile([P, KC_D, P], bf, tag="ctxm", name=f"ctxm_{ci}")
    for c in range(KC_D):
        pt = pps_t.tile([P, P], bf, tag="pt", name=f"pt_{ci}_{c}")
        nc.tensor.transpose(pt[:], ctx[:, c * P:(c + 1) * P], ident[:])
        nc.vector.tensor_copy(ctxm[:, c, :], pt[:])
    pf = pps_c.tile([P, C], f32, tag="pf", name=f"pf_{ci}")
    for c in range(KC_D):
        nc.tensor.matmul(pf[:], ctxm[:, c, :], weff_sb[:, c, :],
                         start=(c == 0), stop=(c == KC_D - 1))
    o_sb = epool.tile([P, C], f32, tag="osb", name=f"osb_{ci}")
    nc.vector.tensor_add(o_sb[:], pf[:], bout_sb[:])
    nc.sync.dma_start(T["o_out"][bsl, :], o_sb[:])
    mx = epool.tile([P, 1], f32, tag="mx")
    nc.vector.reduce_max(mx[:], o_sb[:], axis=AX.X)
    nmx = epool.tile([P, 1], f32, tag="nmx")
    nc.vector.tensor_scalar_mul(nmx[:], mx[:], -1.0)
    esb = epool.tile([P, C], f32, tag="esb")
    nc.scalar.activation(esb[:], o_sb[:], AF.Exp, bias=nmx[:])
    ssb = epool.tile([P, 1], f32, tag="ssb")
    nc.vector.reduce_sum(ssb[:], esb[:], axis=AX.X)
    rsb = epool.tile([P, 1], f32, tag="rsb")
    nc.vector.reciprocal(rsb[:], ssb[:])
    smsb = epool.tile([P, C], f32, tag="smsb")
    nc.vector.tensor_mul(smsb[:], esb[:], rsb[:].broadcast_to([P, C]))
    nc.sync.dma_start(T["sm_out"][bsl, :], smsb[:])


def _load_phase_b_consts(nc, tc, T):
    """Constant pool for phase B, opened before phase A so the weight DMAs
    overlap the GRU instead of serializing at the phase boundary."""
    cpool_cm = tc.tile_pool(name="gb_c", bufs=1)
    cpool = cpool_cm.__enter__()
    wq8_sb = cpool.tile([P, KC_D, D], f8, tag="wq8")
    nc.sync.dma_start(wq8_sb[:], _r3(T["wq8"][:], "(c k) n -> k c n", k=P))
    wk8_sb = cpool.tile([P, KC_D, D], f8, tag="wk8")
    nc.sync.dma_start(wk8_sb[:], _r3(T["wk8"][:], "(c k) n -> k c n", k=P))
    weff_sb = cpool.tile([P, KC_D, C], bf, tag="weff")
    nc.sync.dma_start(weff_sb[:], _r3(T["weff"][:], "(c k) n -> k c n", k=P))
    bout_sb = cpool.tile([P, C], f32, tag="bout")
    nc.sync.dma_start(bout_sb[:], T["bout"][:].to_broadcast([P, C]))
    ident = cpool.tile([P, P], bf, tag="ident")
    make_identity(nc, ident[:])
    return cpool_cm, (wq8_sb, wk8_sb, weff_sb, bout_sb, ident)


def _phase_b(nc, tc, T, hs_bf, consts):
    with (tc.tile_pool(name="gb_h", bufs=3) as hpool,
          tc.tile_pool(name="gb_h8", bufs=1) as h8pool,
          tc.tile_pool(name="gb_wv", bufs=2) as wvpool,
          tc.tile_pool(name="gb_qk", bufs=2) as qkpool,
          tc.tile_pool(name="gb_v", bufs=2) as vpool,
          tc.tile_pool(name="gb_ap", bufs=1) as appool,
          tc.tile_pool(name="gb_sm", bufs=1) as smpool,
          tc.tile_pool(name="gb_tm", bufs=1) as tmpool,
          tc.tile_pool(name="gb_ctx", bufs=8) as ctxpool,
          tc.tile_pool(name="gb_cm", bufs=1) as cmpool,
          tc.tile_pool(name="gb_e", bufs=2) as epool,
          tc.tile_pool(name="gb_pqk", bufs=4, space="PSUM") as pps_qk,
          tc.tile_pool(name="gb_pv", bufs=2, space="PSUM") as pps_v,
          tc.tile_pool(name="gb_pt", bufs=1, space="PSUM") as pps_t,
          tc.tile_pool(name="gb_pc", bufs=1, space="PSUM") as pps_c):
        pools = (hpool, h8pool, wvpool, qkpool, vpool, appool, smpool,
                 tmpool, ctxpool, cmpool, epool, pps_qk, pps_v, pps_t, pps_c)
        ctxs = []
        for pair in range(NCH // 2):
            datas = [_attn_qk(nc, pools, consts, T, hs_bf, ci)
                     for ci in (2 * pair, 2 * pair + 1)]
            _attn_v_pair(nc, pools, T, datas)
            ctxs.extend(_attn_softmax(nc, pools, d_) for d_ in datas)
        for ci in range(NCH):
            _attn_tail(nc, pools, consts, T, ci, ctxs[ci])


def build_nc(reps=1, phases="ab"):
    nc = bacc.Bacc("TRN2", target_bir_lowering=False, debug=False,
                   num_devices=N_CORES, dynamic_dma_scratch_size=8192)

    T = {}
    T["xt"] = nc.dram_tensor("xt", [S, I, B_LOC], bf, kind="ExternalInput")
    for d in ("f", "b"):
        T[f"wcat_{d}"] = nc.dram_tensor(f"wcat_{d}", [2 * H, 3 * H], bf,
                                        kind="ExternalInput")
        T[f"brz_{d}"] = nc.dram_tensor(f"brz_{d}", [2 * H, 1], f32,
                                       kind="ExternalInput")
        T[f"negbz_{d}"] = nc.dram_tensor(f"negbz_{d}", [H, 1], f32,
                                         kind="ExternalInput")
        T[f"bnih_{d}"] = nc.dram_tensor(f"bnih_{d}", [H, 1], f32,
                                        kind="ExternalInput")
        T[f"bnhh_{d}"] = nc.dram_tensor(f"bnhh_{d}", [H, 1], f32,
                                        kind="ExternalInput")
    T["wq8"] = nc.dram_tensor("wq8", [D, D], f8, kind="ExternalInput")
    T["wk8"] = nc.dram_tensor("wk8", [D, D], f8, kind="ExternalInput")
    T["wv"] = nc.dram_tensor("wv", [D, D], bf, kind="ExternalInput")
    T["weff"] = nc.dram_tensor("weff", [D, C], bf, kind="ExternalInput")
    T["bout"] = nc.dram_tensor("bout", [1, C], f32, kind="ExternalInput")
    T["o_out"] = nc.dram_tensor("o_out", [B_LOC, C], f32, kind="ExternalOutput")
    T["sm_out"] = nc.dram_tensor("sm_out", [B_LOC, C], f32,
                                 kind="ExternalOutput")

    with tile.TileContext(nc) as tc:
        for _rep in range(reps):
            with tc.tile_pool(name="dram", bufs=1, space="DRAM") as dram:
                hs_bf = dram.tile([S, D, B_LOC], bf)
                cpool, consts = _load_phase_b_consts(nc, tc, T)
                for ph in phases:
                    if ph == "a":
                        _phase_a(nc, tc, T, hs_bf)
                    elif ph == "b":
                        _phase_b(nc, tc, T, hs_bf, consts)
                cpool.__exit__(None, None, None)

    nc.compile()
    return nc


def _prep_inputs(inputs):
    import ml_dtypes
    npf32 = np.float32
    npbf = ml_dtypes.bfloat16
    npf8 = ml_dtypes.float8_e4m3
    xs = np.stack([np.asarray(inputs["x1"], npf32),
                   np.asarray(inputs["x2"], npf32),
                   np.asarray(inputs["x3"], npf32)])  # (3, B, I)
    shared = {}
    for d in ("f", "b"):
        wih = np.asarray(inputs[f"W_ih_{d}"], npf32)
        whh = np.asarray(inputs[f"W_hh_{d}"], npf32)
        bih = np.asarray(inputs[f"b_ih_{d}"], npf32)
        bhh = np.asarray(inputs[f"b_hh_{d}"], npf32)
        wc = np.concatenate([wih.T, whh.T], axis=0)  # (2I, 3H)
        cols = []
        for j in range(HJT):
            for g in range(3):
                cols.append(wc[:, (g * H + j * P):(g * H + (j + 1) * P)])
        shared[f"wcat_{d}"] = np.ascontiguousarray(
            np.concatenate(cols, axis=1)).astype(npbf)
        bsum = bih + bhh
        shared[f"brz_{d}"] = np.ascontiguousarray(bsum[:2 * H, None])
        shared[f"negbz_{d}"] = np.ascontiguousarray(-bsum[H:2 * H, None])
        shared[f"bnih_{d}"] = np.ascontiguousarray(bih[2 * H:, None])
        shared[f"bnhh_{d}"] = np.ascontiguousarray(bhh[2 * H:, None])
    shared["wq8"] = np.ascontiguousarray(
        np.asarray(inputs["Wq"], npf32).T * W8_SCALE).astype(npf8)
    shared["wk8"] = np.ascontiguousarray(
        np.asarray(inputs["Wk"], npf32).T * W8_SCALE).astype(npf8)
    shared["wv"] = np.ascontiguousarray(
        np.asarray(inputs["Wv"], npf32).T).astype(npbf)
    weff = (np.asarray(inputs["W_out"], np.float64)
            @ np.asarray(inputs["Wo"], np.float64)).T
    shared["weff"] = np.ascontiguousarray(weff.astype(npf32)).astype(npbf)
    shared["bout"] = np.ascontiguousarray(
        np.asarray(inputs["b_out"], npf32)[None, :])

    in_maps = []
    for c in range(N_CORES):
        rows = slice(c * B_LOC, (c + 1) * B_LOC)
        m = dict(shared)
        m["xt"] = np.ascontiguousarray(
            xs[:, rows, :].transpose(0, 2, 1)).astype(npbf)
        in_maps.append(m)
    return in_maps


def _get_nc():
    if "nc" not in _CACHE:
        _CACHE["nc"] = build_nc()
    return _CACHE["nc"]


def kernel(**inputs):
    from concourse.bass_utils import run_bass_kernel_spmd

    nc = _get_nc()
    in_maps = _prep_inputs(inputs)
    res = run_bass_kernel_spmd(nc, in_maps, core_ids=list(range(N_CORES)))
    o = np.concatenate([res.results[c]["o_out"] for c in range(N_CORES)], axis=0)
    sm = np.concatenate([res.results[c]["sm_out"] for c in range(N_CORES)], axis=0)
    return o, sm



# revision 4
# speedup vs baseline: 1.0836x; 1.0836x over previous
"""HELMo encoder (bi-GRU over 3 steps + MHA + classifier) on 8 trn2 cores.

Data-parallel over batch (8192 -> 8 x 1024). Per core, one Bass/Tile kernel.

Key structure (v2):
  A) fused GRU in bf16: input+hidden projections accumulate into f32 PSUM
     (k = [x; h_prev] vs W_cat = [W_ih.T; W_hh.T]); gates split across
     ACT (sigmoid/tanh), DVE (psum-side adds/muls) and Pool (SBUF-side
     combine); h stored bf16, feature-major, and kept entirely in SBUF
     (no DRAM round-trip).
     Per (step, dir), immediately after the cells:
       - U-fold matmuls: because the model sums attention output over the
         3 positions into a 7-class head, o = sum_k sum_h g[b,h,k] *
         (hs_k @ M_h) with M_h = Wv.T[:,h] @ (W_out @ Wo).T[h,:] folded on
         the host (D x NH*C = 2048 x 112). This removes the entire V
         projection, ctx transposes and classifier from the device.
       - fp8 quantization of h (x4) into a resident hsT8 (feature-major),
         consumed by the Q/K projections in phase B.
  B) per 128-row batch chunk, all-SBUF:
     - Q/K projections as fp8e4 DoubleRow matmuls (2x PE rate); weights
       pre-scaled x16, hs x4, compensation folded into the softmax exp
       scale; q/k copied to bf16.
     - logits as fused scalar_tensor_tensor per (head, tq, tk): mul +
       free-dim accumulate in one DVE op at the 4x (2-byte packed SBUF)
       rate.
     - softmax on ACT/DVE; key weights g = sum_q softmax weights.
     - output o = sum_{k,h} g * U (tiny DVE ops) + bias; softmax tail.

Matmul accumulation is f32 PSUM throughout; rel err vs f32 reference
~4e-3, dominated by bf16 weight quantization.
"""

import os
import sys

sys.path.insert(0, "/opt/trn_rl_repo")

import numpy as np

import concourse.bacc as bacc
import concourse.bass as bass
import concourse.mybir as mybir
import concourse.tile as tile
from concourse.masks import make_identity

dt = mybir.dt
AF = mybir.ActivationFunctionType
AX = mybir.AxisListType
PM = mybir.MatmulPerfMode
ALU = mybir.AluOpType

N_CORES = 8
B = 8192
B_LOC = B // N_CORES          # 1024
I = 1024
H = 1024
D = 2 * H                     # 2048
NH = 16
HD = 128
S = 3
C = 7
NC7 = NH * C                  # 112
P = 128
HJT = H // P                  # 8 jtiles per gate
KC_D = D // P                 # 16
NCH = B_LOC // P              # 8 batch chunks in the attention pass

W8_SCALE = 16.0               # fp8 weight pre-scale (host side)
H8_SCALE = 4.0                # fp8 activation pre-scale (device side)
EXP_SCALE = 1.0 / (W8_SCALE * W8_SCALE * H8_SCALE * H8_SCALE * HD ** 0.5)

f32, bf, f8 = dt.float32, dt.bfloat16, dt.float8e4

_CACHE = {}


def _r3(ap, pat, **kw):
    return ap.rearrange(pat, **kw)


def _gru_cell(nc, pps_a, pps_b, gpool, tpool, bias_t, j, bs, first,
              x_chunk, h_prev, h_new, wr, wz, wn):
    t_brz, t_nbz, t_bni, t_bnh = bias_t
    if first:
        pz = pps_b.tile([P, 512], f32, tag="pz")
        for c in range(HJT):
            nc.tensor.matmul(pz[:], wz[:, c, :], x_chunk(c)[:, bs],
                             start=(c == 0), stop=(c == HJT - 1))
        pgi = pps_b.tile([P, 512], f32, tag="pgi")
        for c in range(HJT):
            nc.tensor.matmul(pgi[:], wn[:, c, :], x_chunk(c)[:, bs],
                             start=(c == 0), stop=(c == HJT - 1))
        zc = gpool.tile([P, 512], f32, tag="z")
        nc.scalar.activation(zc[:], pz[:], AF.Sigmoid,
                             bias=t_nbz[:, j, :], scale=-1.0)
        n_sb = gpool.tile([P, 512], f32, tag="n")
        nc.scalar.activation(n_sb[:], pgi[:], AF.Tanh, bias=t_bni[:, j, :])
        nc.gpsimd.tensor_mul(h_new[:, j, bs], zc[:], n_sb[:])
        return

    nk = 2 * HJT

    def mm_acc(ptile, ws):
        for c in range(nk):
            rhs = (x_chunk(c)[:, bs] if c < HJT else h_prev[:, c - HJT, bs])
            nc.tensor.matmul(ptile[:], ws[:, c, :], rhs,
                             start=(c == 0), stop=(c == nk - 1))

    pr = pps_a.tile([P, 512], f32, tag="pr")
    mm_acc(pr, wr)
    pz = pps_b.tile([P, 512], f32, tag="pz")
    mm_acc(pz, wz)
    pgi = pps_b.tile([P, 512], f32, tag="pgi")
    for c in range(HJT):
        nc.tensor.matmul(pgi[:], wn[:, c, :], x_chunk(c)[:, bs],
                         start=(c == 0), stop=(c == HJT - 1))
    pgh = pps_a.tile([P, 512], f32, tag="pgh")
    for c in range(HJT, 2 * HJT):
        nc.tensor.matmul(pgh[:], wn[:, c, :], h_prev[:, c - HJT, bs],
                         start=(c == HJT), stop=(c == 2 * HJT - 1))
    r_sb = gpool.tile([P, 512], f32, tag="r")
    nc.scalar.activation(r_sb[:], pr[:], AF.Sigmoid, bias=t_brz[:, j, :])
    z_sb = gpool.tile([P, 512], f32, tag="z")
    nc.scalar.activation(z_sb[:], pz[:], AF.Sigmoid, bias=t_brz[:, HJT + j, :])
    t1 = tpool.tile([P, 512], f32, tag="tmp")
    nc.vector.tensor_scalar_add(t1[:], pgh[:], t_bnh[:, j, :])
    t2 = tpool.tile([P, 512], f32, tag="tmp")
    nc.vector.tensor_mul(t2[:], r_sb[:], t1[:])
    t3 = tpool.tile([P, 512], f32, tag="tmp")
    nc.vector.tensor_add(t3[:], pgi[:], t2[:])
    n_sb = gpool.tile([P, 512], f32, tag="n")
    nc.scalar.activation(n_sb[:], t3[:], AF.Tanh, bias=t_bni[:, j, :])
    t4 = tpool.tile([P, 512], bf, tag="tmpb")
    nc.gpsimd.tensor_sub(t4[:], h_prev[:, j, bs], n_sb[:])
    t5 = tpool.tile([P, 512], bf, tag="tmpb")
    nc.gpsimd.tensor_mul(t5[:], z_sb[:], t4[:])
    nc.gpsimd.tensor_add(h_new[:, j, bs], t5[:], n_sb[:])


def _phase_a(nc, tc, T, hsT8, ut_sb, u_sb, m_sb, ident):
    with (tc.tile_pool(name="ga_const", bufs=1) as cpool,
          tc.tile_pool(name="ga_x", bufs=3) as xpool,
          tc.tile_pool(name="ga_h", bufs=3) as hpool,
          tc.tile_pool(name="ga_w", bufs=2) as wpool,
          tc.tile_pool(name="ga_g", bufs=2) as gpool,
          tc.tile_pool(name="ga_t", bufs=3) as tpool,
          tc.tile_pool(name="ga_psa", bufs=1, space="PSUM") as pps_a,
          tc.tile_pool(name="ga_psb", bufs=2, space="PSUM") as pps_b,
          tc.tile_pool(name="ga_psu", bufs=2, space="PSUM") as pps_ut):
        bias = {}
        for d in ("f", "b"):
            t_brz = cpool.tile([P, 2 * HJT, 1], f32, tag=f"brz{d}")
            nc.sync.dma_start(t_brz[:], _r3(T[f"brz_{d}"][:], "(c k) o -> k c o", k=P))
            t_nbz = cpool.tile([P, HJT, 1], f32, tag=f"nbz{d}")
            nc.sync.dma_start(t_nbz[:], _r3(T[f"negbz_{d}"][:], "(c k) o -> k c o", k=P))
            t_bni = cpool.tile([P, HJT, 1], f32, tag=f"bni{d}")
            nc.sync.dma_start(t_bni[:], _r3(T[f"bnih_{d}"][:], "(c k) o -> k c o", k=P))
            t_bnh = cpool.tile([P, HJT, 1], f32, tag=f"bnh{d}")
            nc.sync.dma_start(t_bnh[:], _r3(T[f"bnhh_{d}"][:], "(c k) o -> k c o", k=P))
            bias[d] = (t_brz, t_nbz, t_bni, t_bnh)

        order = [(0, "f", 0), (0, "b", 2), (1, "f", 1),
                 (1, "b", 1), (2, "f", 2), (2, "b", 0)]
        h_cur = {"f": None, "b": None}
        ut_started = set()
        last_x = (None, None)
        for step, d, t in order:
            first = step == 0
            if last_x[0] == t:
                x_halves = last_x[1]
            else:
                x_halves = []
                for xh in range(2):
                    xv = xpool.tile([P, HJT // 2, B_LOC], bf, tag="x",
                                    name=f"x_{step}_{d}_{xh}")
                    nc.sync.dma_start(
                        xv[:],
                        _r3(T["xt"][t][xh * (I // 2):(xh + 1) * (I // 2), :],
                            "(c k) b -> k c b", k=P))
                    x_halves.append(xv)
                last_x = (t, x_halves)

            def x_chunk(c):
                return x_halves[c // (HJT // 2)][:, c % (HJT // 2), :]

            h_prev = h_cur[d]
            h_new = hpool.tile([P, HJT, B_LOC], bf, tag="h",
                               name=f"h_{step}_{d}")
            for j in range(HJT):
                # host pre-permutes wcat columns: per j the r/z/n gate
                # columns are adjacent -> one contiguous 384-col DMA
                nkc = HJT if first else 2 * HJT
                wj = wpool.tile([P, nkc, 3 * P], bf, tag="wj",
                                name=f"wj_{step}_{d}_{j}")
                nc.sync.dma_start(
                    wj[:],
                    _r3(T[f"wcat_{d}"][:nkc * P, j * 3 * P:(j + 1) * 3 * P],
                        "(c k) m -> k c m", k=P))
                wr = wj[:, :, 0:P]
                wz = wj[:, :, P:2 * P]
                wn = wj[:, :, 2 * P:3 * P]
                for bt in range(2):
                    bs = slice(bt * 512, (bt + 1) * 512)
                    _gru_cell(nc, pps_a, pps_b, gpool, tpool, bias[d], j, bs,
                              first, x_chunk, h_prev, h_new, wr, wz, wn)
            h_cur[d] = h_new

            # U-fold contribution of this (step, dir): UT[:, t, :] +=
            # M[dir-half].T @ h_new  (k-tiles j=0..7 of this direction).
            koff = 0 if d == "f" else HJT
            for bt in range(2):
                bs = slice(bt * 512, (bt + 1) * 512)
                pu = pps_ut.tile([NC7, 512], f32, tag="ut",
                                 name=f"ut_{step}_{d}_{bt}")
                for j in range(HJT):
                    nc.tensor.matmul(pu[:], m_sb[:, koff + j, :],
                                     h_new[:, j, bs],
                                     start=(j == 0), stop=(j == HJT - 1))
                if t in ut_started:
                    nc.vector.tensor_add(ut_sb[:, t, bs], ut_sb[:, t, bs], pu[:])
                else:
                    nc.vector.tensor_copy(ut_sb[:, t, bs], pu[:])
            ut_started.add(t)

            # fp8 quantization of h for the phase-B Q/K projections
            nc.scalar.activation(hsT8[t][:, koff:koff + HJT, :], h_new[:],
                                 AF.Copy, scale=H8_SCALE)

    # A.5: transpose UT (feature-major) into U (batch-major) for the
    # per-chunk output mix in phase B.
    with tc.tile_pool(name="ga5_ps", bufs=2, space="PSUM") as pps_t:
        for t in range(S):
            for ci in range(NCH):
                pt = pps_t.tile([P, NC7], bf, tag="pt", name=f"pt_{t}_{ci}")
                nc.tensor.transpose(
                    pt[:], ut_sb[:, t, ci * P:(ci + 1) * P],
                    ident[0:NC7, 0:NC7])
                nc.vector.tensor_copy(u_sb[:, ci, t, :], pt[:])


def _phase_b(nc, tc, T, hsT8, u_sb, bout_sb):
    with (tc.tile_pool(name="gb_c", bufs=1) as cpool,
          tc.tile_pool(name="gb_qk", bufs=2) as qkpool,
          tc.tile_pool(name="gb_j", bufs=2) as jpool,
          tc.tile_pool(name="gb_sm", bufs=2) as smpool,
          tc.tile_pool(name="gb_e", bufs=2) as epool,
          tc.tile_pool(name="gb_pqk", bufs=4, space="PSUM") as pps_qk):
        wq8_sb = cpool.tile([P, KC_D, D], f8, tag="wq8")
        nc.sync.dma_start(wq8_sb[:], _r3(T["wq8"][:], "(c k) n -> k c n", k=P))
        wk8_sb = cpool.tile([P, KC_D, D], f8, tag="wk8")
        nc.sync.dma_start(wk8_sb[:], _r3(T["wk8"][:], "(c k) n -> k c n", k=P))

        for ci in range(NCH):
            bsl = slice(ci * P, (ci + 1) * P)
            qt, kt = [], []
            for t in range(S):
                for wsb, lst, nm in ((wq8_sb, qt, "q"), (wk8_sb, kt, "k")):
                    osb = qkpool.tile([P, D], bf, tag=f"{nm}{t}",
                                      name=f"{nm}{t}_{ci}")
                    for do_ in range(4):
                        po = pps_qk.tile([P, 512], f32, tag="pqk",
                                         name=f"p{nm}_{t}_{ci}_{do_}")
                        for c in range(0, KC_D, 2):
                            nc.tensor.matmul(
                                po[:], hsT8[t][:, c:c + 2, bsl],
                                wsb[:, c:c + 2, do_ * 512:(do_ + 1) * 512],
                                start=(c == 0), stop=(c == KC_D - 2),
                                perf_mode=PM.DoubleRow)
                        dsl = slice(do_ * 512, (do_ + 1) * 512)
                        nc.scalar.copy(osb[:, dsl], po[:])
                    lst.append(osb)

            # logits: fused mul+accumulate per (head, tq, tk) on DVE (4x
            # rate for packed bf16 SBUF operands)
            L = smpool.tile([P, NH, S, S], f32, tag="L", name=f"L_{ci}")
            for tq in range(S):
                for tk in range(S):
                    for h in range(NH):
                        hsl = slice(h * HD, (h + 1) * HD)
                        jk = jpool.tile([P, HD], bf, tag="junk")
                        nc.vector.scalar_tensor_tensor(
                            jk[:], qt[tq][:, hsl], 1.0, kt[tk][:, hsl],
                            ALU.mult, ALU.mult,
                            accum_out=L[:, h, tq, tk:tk + 1])

            # softmax (logits are O(0.1) after descaling: exp cannot
            # overflow, skip max-sub). APs kept at <=2 free dims.
            E2 = smpool.tile([P, NH, S, S], f32, tag="E2")
            nc.scalar.activation(_r3(E2[:], "p h q k -> p (h q k)"),
                                 _r3(L[:], "p h q k -> p (h q k)"),
                                 AF.Exp, scale=EXP_SCALE)
            Ssum = smpool.tile([P, NH, S], f32, tag="Ssum")
            nc.vector.reduce_sum(_r3(Ssum[:], "p h q -> p (h q)"),
                                 _r3(E2[:], "p h q k -> p (h q) k"), axis=AX.X)
            Rs = smpool.tile([P, NH, S], f32, tag="Rs")
            nc.vector.reciprocal(_r3(Rs[:], "p h q -> p (h q)"),
                                 _r3(Ssum[:], "p h q -> p (h q)"))
            Wn = smpool.tile([P, NH, S, S], f32, tag="Wn")
            rs_flat = _r3(Rs[:], "p h q -> p (h q)")
            nc.vector.tensor_mul(_r3(Wn[:], "p h q k -> p (h q) k"),
                                 _r3(E2[:], "p h q k -> p (h q) k"),
                                 rs_flat[:, :, None].broadcast_to([P, NH * S, S]))
            wsum = smpool.tile([P, NH, S], f32, tag="wsum")
            wsA = smpool.tile([P, NH, S], f32, tag="wsA")
            nc.vector.tensor_add(wsA[:], Wn[:, :, 0, :], Wn[:, :, 1, :])
            nc.vector.tensor_add(wsum[:], wsA[:], Wn[:, :, 2, :])

            # output mix: o[b, c] = sum_t sum_h wsum[b,h,t] * U[b,t,h,c]
            acc = smpool.tile([P, NH, C], f32, tag="acc")
            u0 = _r3(u_sb[:, ci, 0, :], "p (h c) -> p h c", h=NH)
            nc.vector.tensor_mul(
                acc[:], u0,
                wsum[:, :, 0][:, :, None].broadcast_to([P, NH, C]))
            for t in (1, 2):
                tmp = smpool.tile([P, NH, C], f32, tag="tmpu")
                ut_ = _r3(u_sb[:, ci, t, :], "p (h c) -> p h c", h=NH)
                nc.vector.tensor_mul(
                    tmp[:], ut_,
                    wsum[:, :, t][:, :, None].broadcast_to([P, NH, C]))
                nc.vector.tensor_add(acc[:], acc[:], tmp[:])

            o_pre = epool.tile([P, C], f32, tag="opre")
            nc.vector.reduce_sum(o_pre[:],
                                 _r3(acc[:], "p h c -> p c h"), axis=AX.X)
            o_sb = epool.tile([P, C], f32, tag="osb", name=f"osb_{ci}")
            nc.vector.tensor_add(o_sb[:], o_pre[:], bout_sb[:])
            nc.sync.dma_start(T["o_out"][bsl, :], o_sb[:])
            mx = epool.tile([P, 1], f32, tag="mx")
            nc.vector.reduce_max(mx[:], o_sb[:], axis=AX.X)
            nmx = epool.tile([P, 1], f32, tag="nmx")
            nc.vector.tensor_scalar_mul(nmx[:], mx[:], -1.0)
            esb = epool.tile([P, C], f32, tag="esb")
            nc.scalar.activation(esb[:], o_sb[:], AF.Exp, bias=nmx[:])
            ssb = epool.tile([P, 1], f32, tag="ssb")
            nc.vector.reduce_sum(ssb[:], esb[:], axis=AX.X)
            rsb = epool.tile([P, 1], f32, tag="rsb")
            nc.vector.reciprocal(rsb[:], ssb[:])
            smsb = epool.tile([P, C], f32, tag="smsb")
            nc.vector.tensor_mul(smsb[:], esb[:], rsb[:].broadcast_to([P, C]))
            nc.sync.dma_start(T["sm_out"][bsl, :], smsb[:])


def build_nc(reps=1, phases="ab"):
    nc = bacc.Bacc("TRN2", target_bir_lowering=False, debug=False,
                   num_devices=N_CORES, dynamic_dma_scratch_size=8192)

    T = {}
    T["xt"] = nc.dram_tensor("xt", [S, I, B_LOC], bf, kind="ExternalInput")
    for d in ("f", "b"):
        T[f"wcat_{d}"] = nc.dram_tensor(f"wcat_{d}", [2 * H, 3 * H], bf,
                                        kind="ExternalInput")
        T[f"brz_{d}"] = nc.dram_tensor(f"brz_{d}", [2 * H, 1], f32,
                                       kind="ExternalInput")
        T[f"negbz_{d}"] = nc.dram_tensor(f"negbz_{d}", [H, 1], f32,
                                         kind="ExternalInput")
        T[f"bnih_{d}"] = nc.dram_tensor(f"bnih_{d}", [H, 1], f32,
                                        kind="ExternalInput")
        T[f"bnhh_{d}"] = nc.dram_tensor(f"bnhh_{d}", [H, 1], f32,
                                        kind="ExternalInput")
    T["wq8"] = nc.dram_tensor("wq8", [D, D], f8, kind="ExternalInput")
    T["wk8"] = nc.dram_tensor("wk8", [D, D], f8, kind="ExternalInput")
    T["m_w"] = nc.dram_tensor("m_w", [D, NC7], bf, kind="ExternalInput")
    T["bout"] = nc.dram_tensor("bout", [1, C], f32, kind="ExternalInput")
    T["o_out"] = nc.dram_tensor("o_out", [B_LOC, C], f32, kind="ExternalOutput")
    T["sm_out"] = nc.dram_tensor("sm_out", [B_LOC, C], f32,
                                 kind="ExternalOutput")

    with tile.TileContext(nc) as tc:
        for _rep in range(reps):
            with tc.tile_pool(name="top_c", bufs=1) as tpc:
                m_sb = tpc.tile([P, KC_D, NC7], bf, tag="m")
                nc.sync.dma_start(m_sb[:], _r3(T["m_w"][:], "(c k) n -> k c n", k=P))
                bout_sb = tpc.tile([P, C], f32, tag="bout")
                nc.sync.dma_start(bout_sb[:], T["bout"][:].to_broadcast([P, C]))
                ident = tpc.tile([P, P], bf, tag="ident")
                make_identity(nc, ident[:])
                hsT8 = [tpc.tile([P, KC_D, B_LOC], f8, tag=f"h8_{t}",
                                 name=f"h8_{t}")
                        for t in range(S)]
                ut_sb = tpc.tile([NC7, S, B_LOC], bf, tag="ut")
                u_sb = tpc.tile([P, NCH, S, NC7], bf, tag="u")
                for ph in phases:
                    if ph == "a":
                        _phase_a(nc, tc, T, hsT8, ut_sb, u_sb, m_sb, ident)
                    elif ph == "b":
                        _phase_b(nc, tc, T, hsT8, u_sb, bout_sb)

    nc.compile()
    return nc


def _prep_inputs(inputs):
    import ml_dtypes
    npf32 = np.float32
    npbf = ml_dtypes.bfloat16
    npf8 = ml_dtypes.float8_e4m3
    xs = np.stack([np.asarray(inputs["x1"], npf32),
                   np.asarray(inputs["x2"], npf32),
                   np.asarray(inputs["x3"], npf32)])  # (3, B, I)
    shared = {}
    for d in ("f", "b"):
        wih = np.asarray(inputs[f"W_ih_{d}"], npf32)
        whh = np.asarray(inputs[f"W_hh_{d}"], npf32)
        bih = np.asarray(inputs[f"b_ih_{d}"], npf32)
        bhh = np.asarray(inputs[f"b_hh_{d}"], npf32)
        wc = np.concatenate([wih.T, whh.T], axis=0)  # (2I, 3H)
        cols = []
        for j in range(HJT):
            for g in range(3):
                cols.append(wc[:, (g * H + j * P):(g * H + (j + 1) * P)])
        shared[f"wcat_{d}"] = np.ascontiguousarray(
            np.concatenate(cols, axis=1)).astype(npbf)
        bsum = bih + bhh
        shared[f"brz_{d}"] = np.ascontiguousarray(bsum[:2 * H, None])
        shared[f"negbz_{d}"] = np.ascontiguousarray(-bsum[H:2 * H, None])
        shared[f"bnih_{d}"] = np.ascontiguousarray(bih[2 * H:, None])
        shared[f"bnhh_{d}"] = np.ascontiguousarray(bhh[2 * H:, None])
    shared["wq8"] = np.ascontiguousarray(
        np.asarray(inputs["Wq"], npf32).T * W8_SCALE).astype(npf8)
    shared["wk8"] = np.ascontiguousarray(
        np.asarray(inputs["Wk"], npf32).T * W8_SCALE).astype(npf8)
    wv_t = np.asarray(inputs["Wv"], np.float64).T  # (D, D)
    weff = (np.asarray(inputs["W_out"], np.float64)
            @ np.asarray(inputs["Wo"], np.float64)).T  # (D, C)
    m_w = np.zeros((D, NC7), np.float64)
    for h in range(NH):
        hs_ = slice(h * HD, (h + 1) * HD)
        m_w[:, h * C:(h + 1) * C] = wv_t[:, hs_] @ weff[hs_, :]
    shared["m_w"] = np.ascontiguousarray(m_w.astype(npf32)).astype(npbf)
    shared["bout"] = np.ascontiguousarray(
        np.asarray(inputs["b_out"], npf32)[None, :])

    in_maps = []
    for c in range(N_CORES):
        rows = slice(c * B_LOC, (c + 1) * B_LOC)
        m = dict(shared)
        m["xt"] = np.ascontiguousarray(
            xs[:, rows, :].transpose(0, 2, 1)).astype(npbf)
        in_maps.append(m)
    return in_maps


def _get_nc():
    if "nc" not in _CACHE:
        _CACHE["nc"] = build_nc()
    return _CACHE["nc"]


def kernel(**inputs):
    from concourse.bass_utils import run_bass_kernel_spmd

    nc = _get_nc()
    in_maps = _prep_inputs(inputs)
    res = run_bass_kernel_spmd(nc, in_maps, core_ids=list(range(N_CORES)))
    o = np.concatenate([res.results[c]["o_out"] for c in range(N_CORES)], axis=0)
    sm = np.concatenate([res.results[c]["sm_out"] for c in range(N_CORES)], axis=0)
    return o, sm


# revision 10
# speedup vs baseline: 1.4348x; 1.3242x over previous
"""HELMo encoder (bi-GRU over 3 steps + MHA + classifier) on 8 trn2 cores.

Data-parallel over batch (8192 -> 8 x 1024). Per core, one Bass/Tile kernel.

Key structure (v2):
  A) fused GRU in bf16: input+hidden projections accumulate into f32 PSUM
     (k = [x; h_prev] vs W_cat = [W_ih.T; W_hh.T]); gates split across
     ACT (sigmoid/tanh), DVE (psum-side adds/muls) and Pool (SBUF-side
     combine); h stored bf16, feature-major, and kept entirely in SBUF
     (no DRAM round-trip).
     Per (step, dir), immediately after the cells:
       - U-fold matmuls: because the model sums attention output over the
         3 positions into a 7-class head, o = sum_k sum_h g[b,h,k] *
         (hs_k @ M_h) with M_h = Wv.T[:,h] @ (W_out @ Wo).T[h,:] folded on
         the host (D x NH*C = 2048 x 112). This removes the entire V
         projection, ctx transposes and classifier from the device.
       - fp8 quantization of h (x4) into a resident hsT8 (feature-major),
         consumed by the Q/K projections in phase B.
  B) per 128-row batch chunk, all-SBUF:
     - Q/K projections as fp8e4 DoubleRow matmuls (2x PE rate); weights
       pre-scaled x16, hs x4, compensation folded into the softmax exp
       scale; q/k copied to bf16.
     - logits as fused scalar_tensor_tensor per (head, tq, tk): mul +
       free-dim accumulate in one DVE op at the 4x (2-byte packed SBUF)
       rate.
     - softmax on ACT/DVE; key weights g = sum_q softmax weights.
     - output o = sum_{k,h} g * U (tiny DVE ops) + bias; softmax tail.

Matmul accumulation is f32 PSUM throughout; rel err vs f32 reference
~4e-3, dominated by bf16 weight quantization.
"""

import os
import sys

sys.path.insert(0, "/opt/trn_rl_repo")

import numpy as np

import concourse.bacc as bacc
import concourse.bass as bass
import concourse.mybir as mybir
import concourse.tile as tile
from concourse.masks import make_identity

dt = mybir.dt
AF = mybir.ActivationFunctionType
AX = mybir.AxisListType
PM = mybir.MatmulPerfMode
ALU = mybir.AluOpType

N_CORES = 8
B = 8192
B_LOC = B // N_CORES          # 1024
I = 1024
H = 1024
D = 2 * H                     # 2048
NH = 16
HD = 128
S = 3
C = 7
NC7 = NH * C                  # 112
P = 128
HJT = H // P                  # 8 jtiles per gate
KC_D = D // P                 # 16
NCH = B_LOC // P              # 8 batch chunks in the attention pass

W8_SCALE = 16.0               # fp8 weight pre-scale (host side)
H8_SCALE = 4.0                # fp8 activation pre-scale (device side)
EXP_SCALE = 1.0 / (W8_SCALE * W8_SCALE * H8_SCALE * H8_SCALE * HD ** 0.5)

f32, bf, f8 = dt.float32, dt.bfloat16, dt.float8e4

_CACHE = {}


def _r3(ap, pat, **kw):
    return ap.rearrange(pat, **kw)


def _gru_cell(nc, pps_a, pps_b, gpool, tpool, bias_t, j, bs, first,
              x_chunk, h_prev, h_new, wr, wz, wn):
    t_brz, t_nbz, t_bni, t_bnh = bias_t
    if first:
        pz = pps_b.tile([P, 512], f32, tag="pz")
        for c in range(HJT):
            nc.tensor.matmul(pz[:], wz[:, c, :], x_chunk(c)[:, bs],
                             start=(c == 0), stop=(c == HJT - 1))
        pgi = pps_b.tile([P, 512], f32, tag="pgi")
        for c in range(HJT):
            nc.tensor.matmul(pgi[:], wn[:, c, :], x_chunk(c)[:, bs],
                             start=(c == 0), stop=(c == HJT - 1))
        zc = gpool.tile([P, 512], f32, tag="z")
        nc.scalar.activation(zc[:], pz[:], AF.Sigmoid,
                             bias=t_nbz[:, j, :], scale=-1.0)
        n_sb = gpool.tile([P, 512], f32, tag="n")
        nc.scalar.activation(n_sb[:], pgi[:], AF.Tanh, bias=t_bni[:, j, :])
        nc.gpsimd.tensor_mul(h_new[:, j, bs], zc[:], n_sb[:])
        return

    nk = 2 * HJT

    def mm_acc(ptile, ws):
        for c in range(nk):
            rhs = (x_chunk(c)[:, bs] if c < HJT else h_prev[:, c - HJT, bs])
            nc.tensor.matmul(ptile[:], ws[:, c, :], rhs,
                             start=(c == 0), stop=(c == nk - 1))

    pr = pps_a.tile([P, 512], f32, tag="pr")
    mm_acc(pr, wr)
    pz = pps_b.tile([P, 512], f32, tag="pz")
    mm_acc(pz, wz)
    pgi = pps_b.tile([P, 512], f32, tag="pgi")
    for c in range(HJT):
        nc.tensor.matmul(pgi[:], wn[:, c, :], x_chunk(c)[:, bs],
                         start=(c == 0), stop=(c == HJT - 1))
    pgh = pps_a.tile([P, 512], f32, tag="pgh")
    for c in range(HJT, 2 * HJT):
        nc.tensor.matmul(pgh[:], wn[:, c, :], h_prev[:, c - HJT, bs],
                         start=(c == HJT), stop=(c == 2 * HJT - 1))
    r_sb = gpool.tile([P, 512], f32, tag="r")
    nc.scalar.activation(r_sb[:], pr[:], AF.Sigmoid, bias=t_brz[:, j, :])
    z_sb = gpool.tile([P, 512], f32, tag="z")
    nc.scalar.activation(z_sb[:], pz[:], AF.Sigmoid, bias=t_brz[:, HJT + j, :])
    t1 = tpool.tile([P, 512], f32, tag="tmp")
    nc.vector.tensor_scalar_add(t1[:], pgh[:], t_bnh[:, j, :])
    t2 = tpool.tile([P, 512], f32, tag="tmp")
    nc.vector.tensor_mul(t2[:], r_sb[:], t1[:])
    t3 = tpool.tile([P, 512], f32, tag="tmp")
    nc.vector.tensor_add(t3[:], pgi[:], t2[:])
    n_sb = gpool.tile([P, 512], f32, tag="n")
    nc.scalar.activation(n_sb[:], t3[:], AF.Tanh, bias=t_bni[:, j, :])
    t4 = tpool.tile([P, 512], bf, tag="tmpb")
    nc.gpsimd.tensor_sub(t4[:], h_prev[:, j, bs], n_sb[:])
    t5 = tpool.tile([P, 512], bf, tag="tmpb")
    nc.gpsimd.tensor_mul(t5[:], z_sb[:], t4[:])
    nc.gpsimd.tensor_add(h_new[:, j, bs], t5[:], n_sb[:])


def _phase_a(nc, tc, T, hsT8, ut_sb, u_sb, m_sb, ident):
    with (tc.tile_pool(name="ga_const", bufs=1) as cpool,
          tc.tile_pool(name="ga_x", bufs=3) as xpool,
          tc.tile_pool(name="ga_h", bufs=3) as hpool,
          tc.tile_pool(name="ga_w", bufs=2) as wpool,
          tc.tile_pool(name="ga_g", bufs=2) as gpool,
          tc.tile_pool(name="ga_t", bufs=3) as tpool,
          tc.tile_pool(name="ga_psa", bufs=1, space="PSUM") as pps_a,
          tc.tile_pool(name="ga_psb", bufs=2, space="PSUM") as pps_b,
          tc.tile_pool(name="ga_psu", bufs=2, space="PSUM") as pps_ut):
        bias = {}
        for d in ("f", "b"):
            t_brz = cpool.tile([P, 2 * HJT, 1], f32, tag=f"brz{d}")
            nc.sync.dma_start(t_brz[:], _r3(T[f"brz_{d}"][:], "(c k) o -> k c o", k=P))
            t_nbz = cpool.tile([P, HJT, 1], f32, tag=f"nbz{d}")
            nc.sync.dma_start(t_nbz[:], _r3(T[f"negbz_{d}"][:], "(c k) o -> k c o", k=P))
            t_bni = cpool.tile([P, HJT, 1], f32, tag=f"bni{d}")
            nc.sync.dma_start(t_bni[:], _r3(T[f"bnih_{d}"][:], "(c k) o -> k c o", k=P))
            t_bnh = cpool.tile([P, HJT, 1], f32, tag=f"bnh{d}")
            nc.sync.dma_start(t_bnh[:], _r3(T[f"bnhh_{d}"][:], "(c k) o -> k c o", k=P))
            bias[d] = (t_brz, t_nbz, t_bni, t_bnh)

        order = [(0, "f", 0), (0, "b", 2), (1, "f", 1),
                 (1, "b", 1), (2, "f", 2), (2, "b", 0)]
        h_cur = {"f": None, "b": None}
        ut_started = set()
        last_x = (None, None)
        for step, d, t in order:
            first = step == 0
            if last_x[0] == t:
                x_halves = last_x[1]
            else:
                x_halves = []
                for xh in range(2):
                    xv = xpool.tile([P, HJT // 2, B_LOC], bf, tag="x",
                                    name=f"x_{step}_{d}_{xh}")
                    nc.sync.dma_start(
                        xv[:],
                        _r3(T["xt"][t][xh * (I // 2):(xh + 1) * (I // 2), :],
                            "(c k) b -> k c b", k=P))
                    x_halves.append(xv)
                last_x = (t, x_halves)

            def x_chunk(c):
                return x_halves[c // (HJT // 2)][:, c % (HJT // 2), :]

            h_prev = h_cur[d]
            h_new = hpool.tile([P, HJT, B_LOC], bf, tag="h",
                               name=f"h_{step}_{d}")
            for j in range(HJT):
                # host pre-permutes wcat columns: per j the r/z/n gate
                # columns are adjacent -> one contiguous 384-col DMA
                nkc = HJT if first else 2 * HJT
                wj = wpool.tile([P, nkc, 3 * P], bf, tag="wj",
                                name=f"wj_{step}_{d}_{j}")
                nc.sync.dma_start(
                    wj[:],
                    _r3(T[f"wcat_{d}"][:nkc * P, j * 3 * P:(j + 1) * 3 * P],
                        "(c k) m -> k c m", k=P))
                wr = wj[:, :, 0:P]
                wz = wj[:, :, P:2 * P]
                wn = wj[:, :, 2 * P:3 * P]
                for bt in range(2):
                    bs = slice(bt * 512, (bt + 1) * 512)
                    _gru_cell(nc, pps_a, pps_b, gpool, tpool, bias[d], j, bs,
                              first, x_chunk, h_prev, h_new, wr, wz, wn)
            h_cur[d] = h_new

            # U-fold contribution of this (step, dir): UT[:, t, :] +=
            # M[dir-half].T @ h_new  (k-tiles j=0..7 of this direction).
            koff = 0 if d == "f" else HJT
            for bt in range(2):
                bs = slice(bt * 512, (bt + 1) * 512)
                pu = pps_ut.tile([NC7, 512], f32, tag="ut",
                                 name=f"ut_{step}_{d}_{bt}")
                for j in range(HJT):
                    nc.tensor.matmul(pu[:], m_sb[:, koff + j, :],
                                     h_new[:, j, bs],
                                     start=(j == 0), stop=(j == HJT - 1))
                if t in ut_started:
                    nc.vector.tensor_add(ut_sb[:, t, bs], ut_sb[:, t, bs], pu[:])
                else:
                    nc.vector.tensor_copy(ut_sb[:, t, bs], pu[:])
            ut_started.add(t)

            # fp8 quantization of h for the phase-B Q/K projections
            nc.scalar.activation(hsT8[t][:, koff:koff + HJT, :], h_new[:],
                                 AF.Copy, scale=H8_SCALE)

    # A.5: transpose UT (feature-major) into U (batch-major) for the
    # per-chunk output mix in phase B.
    with tc.tile_pool(name="ga5_ps", bufs=2, space="PSUM") as pps_t:
        for t in range(S):
            for ci in range(NCH):
                pt = pps_t.tile([P, NC7], bf, tag="pt", name=f"pt_{t}_{ci}")
                nc.tensor.transpose(
                    pt[:], ut_sb[:, t, ci * P:(ci + 1) * P],
                    ident[0:NC7, 0:NC7])
                nc.vector.tensor_copy(u_sb[:, ci, t, :], pt[:])


def _phase_b(nc, tc, T, hsT8, u_sb, bout_sb, parts="qklm"):
    G = 2  # batch chunks per group: fatter DVE/ACT ops, fewer sem waits
    with (tc.tile_pool(name="gb_c", bufs=1) as cpool,
          tc.tile_pool(name="gb_qk", bufs=2) as qkpool,
          tc.tile_pool(name="gb_pr", bufs=2) as prpool,
          tc.tile_pool(name="gb_sm", bufs=2) as smpool,
          tc.tile_pool(name="gb_e", bufs=2) as epool,
          tc.tile_pool(name="gb_pqk", bufs=4, space="PSUM") as pps_qk):
        wq8_sb = cpool.tile([P, KC_D, D], f8, tag="wq8")
        nc.sync.dma_start(wq8_sb[:], _r3(T["wq8"][:], "(c k) n -> k c n", k=P))
        wk8_sb = cpool.tile([P, KC_D, D], f8, tag="wk8")
        nc.sync.dma_start(wk8_sb[:], _r3(T["wk8"][:], "(c k) n -> k c n", k=P))

        if "q" not in parts:
            qfix, kfix = [], []
            for t in range(S):
                for lst, nm in ((qfix, "qf"), (kfix, "kf")):
                    osb = cpool.tile([P, G, D], f8, tag=f"{nm}{t}",
                                     name=f"{nm}{t}_fix")
                    nc.gpsimd.memset(osb[:], 0)
                    lst.append(osb)

        for g in range(NCH // G):
            if "q" in parts:
                qt, kt = [], []
                for t in range(S):
                    for wsb, lst, nm in ((wq8_sb, qt, "q"), (wk8_sb, kt, "k")):
                        osb = qkpool.tile([P, G, D], f8, tag=f"{nm}{t}",
                                          name=f"{nm}{t}_{g}")
                        for gi in range(G):
                            bsl = slice((g * G + gi) * P, (g * G + gi + 1) * P)
                            for do_ in range(4):
                                po = pps_qk.tile([P, 512], f32, tag="pqk",
                                                 name=f"p{nm}_{t}_{g}_{gi}_{do_}")
                                for c in range(0, KC_D, 2):
                                    nc.tensor.matmul(
                                        po[:], hsT8[t][:, c:c + 2, bsl],
                                        wsb[:, c:c + 2, do_ * 512:(do_ + 1) * 512],
                                        start=(c == 0), stop=(c == KC_D - 2),
                                        perf_mode=PM.DoubleRow)
                                dsl = slice(do_ * 512, (do_ + 1) * 512)
                                nc.scalar.copy(osb[:, gi, dsl], po[:])
                        lst.append(osb)
            else:
                qt, kt = qfix, kfix
            if "l" not in parts:
                continue

            # logits: per (tq, tk) one fat product + one segmented reduce
            L = smpool.tile([P, G, NH, S, S], f32, tag="L", name=f"L_{g}")
            for tq in range(S):
                for tk in range(S):
                    pr = prpool.tile([P, G, D], bf, tag="pr",
                                     name=f"pr_{g}_{tq}_{tk}")
                    nc.vector.tensor_mul(pr[:], qt[tq][:], kt[tk][:])
                    nc.vector.reduce_sum(
                        _r3(L[:, :, :, tq, tk], "p g h -> p (g h)"),
                        _r3(pr[:], "p g (h e) -> p (g h) e", h=NH), axis=AX.X)

            if "m" not in parts:
                continue
            # softmax (logits are O(0.1) after descaling: exp cannot
            # overflow, skip max-sub)
            E2 = smpool.tile([P, G, NH, S, S], f32, tag="E2")
            nc.scalar.activation(_r3(E2[:], "p g h q k -> p (g h q k)"),
                                 _r3(L[:], "p g h q k -> p (g h q k)"),
                                 AF.Exp, scale=EXP_SCALE)
            Ssum = smpool.tile([P, G, NH, S], f32, tag="Ssum")
            nc.vector.reduce_sum(_r3(Ssum[:], "p g h q -> p (g h q)"),
                                 _r3(E2[:], "p g h q k -> p (g h q) k"),
                                 axis=AX.X)
            Rs = smpool.tile([P, G, NH, S], f32, tag="Rs")
            nc.vector.reciprocal(_r3(Rs[:], "p g h q -> p (g h q)"),
                                 _r3(Ssum[:], "p g h q -> p (g h q)"))
            Wn = smpool.tile([P, G, NH, S, S], f32, tag="Wn")
            rs_flat = _r3(Rs[:], "p g h q -> p (g h q)")
            nc.vector.tensor_mul(
                _r3(Wn[:], "p g h q k -> p (g h q) k"),
                _r3(E2[:], "p g h q k -> p (g h q) k"),
                rs_flat[:, :, None].broadcast_to([P, G * NH * S, S]))
            wsum = smpool.tile([P, G, NH, S], f32, tag="wsum")
            wsA = smpool.tile([P, G, NH, S], f32, tag="wsA")
            nc.vector.tensor_add(_r3(wsA[:], "p g h k -> p (g h) k"),
                                 _r3(Wn[:, :, :, 0, :], "p g h k -> p (g h) k"),
                                 _r3(Wn[:, :, :, 1, :], "p g h k -> p (g h) k"))
            nc.vector.tensor_add(_r3(wsum[:], "p g h k -> p (g h) k"),
                                 _r3(wsA[:], "p g h k -> p (g h) k"),
                                 _r3(Wn[:, :, :, 2, :], "p g h k -> p (g h) k"))

            # output mix: o[b, c] = sum_t sum_h wsum[b,h,t] * U[b,t,h,c]
            acc = smpool.tile([P, G, NH, C], f32, tag="acc")
            u0 = _r3(u_sb[:, g * G:(g + 1) * G, 0, :],
                     "p g (h c) -> p g h c", h=NH)
            nc.vector.tensor_mul(
                acc[:], u0,
                wsum[:, :, :, 0][:, :, :, None].broadcast_to([P, G, NH, C]))
            for t in (1, 2):
                tmp = smpool.tile([P, G, NH, C], f32, tag="tmpu")
                ut_ = _r3(u_sb[:, g * G:(g + 1) * G, t, :],
                          "p g (h c) -> p g h c", h=NH)
                nc.vector.tensor_mul(
                    tmp[:], ut_,
                    wsum[:, :, :, t][:, :, :, None].broadcast_to([P, G, NH, C]))
                nc.vector.tensor_add(acc[:], acc[:], tmp[:])

            o_pre = epool.tile([P, G, C], f32, tag="opre")
            nc.vector.reduce_sum(o_pre[:],
                                 _r3(acc[:], "p g h c -> p g c h"),
                                 axis=AX.X)
            o_sb = epool.tile([P, G, C], f32, tag="osb", name=f"osb_{g}")
            nc.vector.tensor_add(
                o_sb[:], o_pre[:],
                bout_sb[:, None, :].broadcast_to([P, G, C]))
            gsl = slice(g * G * P, (g + 1) * G * P)
            nc.sync.dma_start(
                _r3(T["o_out"][gsl, :], "(g k) c -> k g c", k=P), o_sb[:])
            # softmax tail without max-sub (|o| is O(1): exp safe in f32)
            esb = epool.tile([P, G, C], f32, tag="esb")
            nc.scalar.activation(_r3(esb[:], "p g c -> p (g c)"),
                                 _r3(o_sb[:], "p g c -> p (g c)"), AF.Exp)
            ssb = epool.tile([P, G, 1], f32, tag="ssb")
            nc.vector.reduce_sum(ssb[:], esb[:], axis=AX.X)
            rsb = epool.tile([P, G, 1], f32, tag="rsb")
            nc.vector.reciprocal(_r3(rsb[:], "p g o -> p (g o)"),
                                 _r3(ssb[:], "p g o -> p (g o)"))
            smsb = epool.tile([P, G, C], f32, tag="smsb")
            nc.vector.tensor_mul(smsb[:], esb[:],
                                 rsb[:].broadcast_to([P, G, C]))
            nc.sync.dma_start(
                _r3(T["sm_out"][gsl, :], "(g k) c -> k g c", k=P), smsb[:])


def build_nc(reps=1, phases="ab"):
    nc = bacc.Bacc("TRN2", target_bir_lowering=False, debug=False,
                   num_devices=N_CORES, dynamic_dma_scratch_size=8192)

    T = {}
    T["xt"] = nc.dram_tensor("xt", [S, I, B_LOC], bf, kind="ExternalInput")
    for d in ("f", "b"):
        T[f"wcat_{d}"] = nc.dram_tensor(f"wcat_{d}", [2 * H, 3 * H], bf,
                                        kind="ExternalInput")
        T[f"brz_{d}"] = nc.dram_tensor(f"brz_{d}", [2 * H, 1], f32,
                                       kind="ExternalInput")
        T[f"negbz_{d}"] = nc.dram_tensor(f"negbz_{d}", [H, 1], f32,
                                         kind="ExternalInput")
        T[f"bnih_{d}"] = nc.dram_tensor(f"bnih_{d}", [H, 1], f32,
                                        kind="ExternalInput")
        T[f"bnhh_{d}"] = nc.dram_tensor(f"bnhh_{d}", [H, 1], f32,
                                        kind="ExternalInput")
    T["wq8"] = nc.dram_tensor("wq8", [D, D], f8, kind="ExternalInput")
    T["wk8"] = nc.dram_tensor("wk8", [D, D], f8, kind="ExternalInput")
    T["m_w"] = nc.dram_tensor("m_w", [D, NC7], bf, kind="ExternalInput")
    T["bout"] = nc.dram_tensor("bout", [1, C], f32, kind="ExternalInput")
    T["o_out"] = nc.dram_tensor("o_out", [B_LOC, C], f32, kind="ExternalOutput")
    T["sm_out"] = nc.dram_tensor("sm_out", [B_LOC, C], f32,
                                 kind="ExternalOutput")

    with tile.TileContext(nc) as tc:
        for _rep in range(reps):
            with tc.tile_pool(name="top_c", bufs=1) as tpc:
                m_sb = tpc.tile([P, KC_D, NC7], bf, tag="m")
                nc.sync.dma_start(m_sb[:], _r3(T["m_w"][:], "(c k) n -> k c n", k=P))
                bout_sb = tpc.tile([P, C], f32, tag="bout")
                nc.sync.dma_start(bout_sb[:], T["bout"][:].to_broadcast([P, C]))
                ident = tpc.tile([P, P], bf, tag="ident")
                make_identity(nc, ident[:])
                hsT8 = [tpc.tile([P, KC_D, B_LOC], f8, tag=f"h8_{t}",
                                 name=f"h8_{t}")
                        for t in range(S)]
                ut_sb = tpc.tile([NC7, S, B_LOC], bf, tag="ut")
                u_sb = tpc.tile([P, NCH, S, NC7], bf, tag="u")
                if "a" not in phases:
                    # timing-only path: make phase-A outputs defined
                    for t in range(S):
                        nc.gpsimd.memset(hsT8[t][:], 0)
                    nc.gpsimd.memset(u_sb[:], 0)
                for ph in phases:
                    if ph == "a":
                        _phase_a(nc, tc, T, hsT8, ut_sb, u_sb, m_sb, ident)
                    elif ph == "b":
                        _phase_b(nc, tc, T, hsT8, u_sb, bout_sb)
                    elif ph == "q":   # QK matmuls + copies only
                        _phase_b(nc, tc, T, hsT8, u_sb, bout_sb, parts="qk")
                    elif ph == "l":   # logits + softmax + mix only
                        _phase_b(nc, tc, T, hsT8, u_sb, bout_sb, parts="lm")

    nc.compile()
    return nc


def _prep_inputs(inputs):
    import ml_dtypes
    npf32 = np.float32
    npbf = ml_dtypes.bfloat16
    npf8 = ml_dtypes.float8_e4m3
    xs = np.stack([np.asarray(inputs["x1"], npf32),
                   np.asarray(inputs["x2"], npf32),
                   np.asarray(inputs["x3"], npf32)])  # (3, B, I)
    shared = {}
    for d in ("f", "b"):
        wih = np.asarray(inputs[f"W_ih_{d}"], npf32)
        whh = np.asarray(inputs[f"W_hh_{d}"], npf32)
        bih = np.asarray(inputs[f"b_ih_{d}"], npf32)
        bhh = np.asarray(inputs[f"b_hh_{d}"], npf32)
        wc = np.concatenate([wih.T, whh.T], axis=0)  # (2I, 3H)
        cols = []
        for j in range(HJT):
            for g in range(3):
                cols.append(wc[:, (g * H + j * P):(g * H + (j + 1) * P)])
        shared[f"wcat_{d}"] = np.ascontiguousarray(
            np.concatenate(cols, axis=1)).astype(npbf)
        bsum = bih + bhh
        shared[f"brz_{d}"] = np.ascontiguousarray(bsum[:2 * H, None])
        shared[f"negbz_{d}"] = np.ascontiguousarray(-bsum[H:2 * H, None])
        shared[f"bnih_{d}"] = np.ascontiguousarray(bih[2 * H:, None])
        shared[f"bnhh_{d}"] = np.ascontiguousarray(bhh[2 * H:, None])
    shared["wq8"] = np.ascontiguousarray(
        np.asarray(inputs["Wq"], npf32).T * W8_SCALE).astype(npf8)
    shared["wk8"] = np.ascontiguousarray(
        np.asarray(inputs["Wk"], npf32).T * W8_SCALE).astype(npf8)
    wv_t = np.asarray(inputs["Wv"], np.float64).T  # (D, D)
    weff = (np.asarray(inputs["W_out"], np.float64)
            @ np.asarray(inputs["Wo"], np.float64)).T  # (D, C)
    m_w = np.zeros((D, NC7), np.float64)
    for h in range(NH):
        hs_ = slice(h * HD, (h + 1) * HD)
        m_w[:, h * C:(h + 1) * C] = wv_t[:, hs_] @ weff[hs_, :]
    shared["m_w"] = np.ascontiguousarray(m_w.astype(npf32)).astype(npbf)
    shared["bout"] = np.ascontiguousarray(
        np.asarray(inputs["b_out"], npf32)[None, :])

    in_maps = []
    for c in range(N_CORES):
        rows = slice(c * B_LOC, (c + 1) * B_LOC)
        m = dict(shared)
        m["xt"] = np.ascontiguousarray(
            xs[:, rows, :].transpose(0, 2, 1)).astype(npbf)
        in_maps.append(m)
    return in_maps


def _get_nc():
    if "nc" not in _CACHE:
        _CACHE["nc"] = build_nc()
    return _CACHE["nc"]


def kernel(**inputs):
    from concourse.bass_utils import run_bass_kernel_spmd

    nc = _get_nc()
    in_maps = _prep_inputs(inputs)
    res = run_bass_kernel_spmd(nc, in_maps, core_ids=list(range(N_CORES)))
    o = np.concatenate([res.results[c]["o_out"] for c in range(N_CORES)], axis=0)
    sm = np.concatenate([res.results[c]["sm_out"] for c in range(N_CORES)], axis=0)
    return o, sm
